# revision 7
# baseline (speedup 1.0000x reference)
"""AERGCN (2-layer R-GCN + bilinear attention pool) on 8 TRN2 NeuronCores.

Sharding: relation-expert. The 41 relations are padded to 48 slots; core c
owns slots [6c, 6c+6) (dummy slots get zero weights/adjacency and an exp-bias
of -1e4 so they vanish from the relation softmax). Each core reads only its
own slice of w_rgcn (the dominant HBM traffic), computes unnormalized
exp-weighted partial sums, and one AllReduce of [num | den] per layer
reconstitutes the softmax-combined hidden state. After layer 2 a
ReduceScatter hands batch c to core c, which runs the attention pool for
that batch alone.

Matmuls run in bf16 (f32 PSUM accumulate). Per-(slot,batch) pipeline:
  hidden = h @ [W_r | W_r @ score_w]          (accumulating matmuls)
  [logun | denom] = adj @ [hsw | ones]        (1 matmul, N=2, lhsT=adjT)
  e = exp(logun/denom + score_b); scr = e/denom
  num[b] += scr * (adj @ hidden)              (2 matmuls N=384 + DVE accum)
"""

import numpy as np
import ml_dtypes

bf16 = ml_dtypes.bfloat16

B, S, F, R, NL = 8, 128, 768, 41, 2
NH, HD, EMB = 8, 96, 768
RLOC, NCORES, IC = 6, 8, 6
FE = F + 1  # 769: W with appended u column
EBIAS_MASK = -1e4

_CACHE = {}


def _build_graph():
    if "nc" in _CACHE:
        return _CACHE["nc"]

    import concourse.mybir as mybir
    import concourse.tile as tile
    from concourse import bacc
    from concourse.masks import make_identity

    dt = mybir.dt
    AF = mybir.ActivationFunctionType
    OP = mybir.AluOpType

    nc = bacc.Bacc("TRN2", target_bir_lowering=False, debug=False,
                   num_devices=NCORES)

    # ---------------- DRAM I/O (per-core shapes) ----------------
    xt = nc.dram_tensor("xt", [B, F, S], dt.bfloat16, kind="ExternalInput")
    adjt = nc.dram_tensor("adjt", [RLOC, B, S, S], dt.bfloat16, kind="ExternalInput")
    w = nc.dram_tensor("w", [NL, RLOC, F, FE], dt.bfloat16, kind="ExternalInput")
    ebias = nc.dram_tensor("ebias", [NL, S, RLOC], dt.float32, kind="ExternalInput")
    wk_d = nc.dram_tensor("wk", [F, F], dt.bfloat16, kind="ExternalInput")
    wq_d = nc.dram_tensor("wq", [F, F], dt.bfloat16, kind="ExternalInput")
    wbil_d = nc.dram_tensor("wbil", [HD, HD], dt.bfloat16, kind="ExternalInput")
    wproj_d = nc.dram_tensor("wproj", [F, F], dt.bfloat16, kind="ExternalInput")
    bk_d = nc.dram_tensor("bk", [1, F], dt.bfloat16, kind="ExternalInput")
    bq_d = nc.dram_tensor("bq", [1, F], dt.bfloat16, kind="ExternalInput")
    bproj_d = nc.dram_tensor("bproj", [1, F], dt.bfloat16, kind="ExternalInput")
    qcol_d = nc.dram_tensor("qcol", [IC, S, 1], dt.bfloat16, kind="ExternalInput")
    out_d = nc.dram_tensor("out", [1, F], dt.float32, kind="ExternalOutput")

    groups = [list(range(NCORES))]

    with tile.TileContext(nc) as tc:
        with (
            tc.tile_pool(name="const", bufs=1) as constp,
            tc.tile_pool(name="wpool", bufs=12) as wpool,
            tc.tile_pool(name="hidp", bufs=16) as hidp,
            tc.tile_pool(name="adjp", bufs=16) as adjp,
            tc.tile_pool(name="hT", bufs=8) as hTp,
            tc.tile_pool(name="payl", bufs=8) as paylp,
            tc.tile_pool(name="tail", bufs=3) as tailp,
            tc.tile_pool(name="misc", bufs=2) as miscp,
            tc.tile_pool(name="dram", bufs=1, space="DRAM") as dramp,
            tc.tile_pool(name="ps_hid", bufs=2, space="PSUM") as ps_hid,
            tc.tile_pool(name="ps_ld", bufs=2, space="PSUM") as ps_ld,
            tc.tile_pool(name="ps_intm", bufs=2, space="PSUM") as ps_intm,
        ):
            ident_b = constp.tile([128, 128], dt.bfloat16, name="ident_b")
            make_identity(nc, ident_b)
            ident_f = constp.tile([128, 128], dt.float32, name="ident_f")
            make_identity(nc, ident_f)
            ones_row = constp.tile([1, 128], dt.bfloat16, name="ones_row")
            nc.vector.memset(ones_row, 1.0)
            one_sb = constp.tile([1, 1], dt.bfloat16, name="one_sb")
            nc.vector.memset(one_sb, 1.0)
            ebias_sb = constp.tile([S, NL * RLOC], dt.float32, name="ebias_sb")
            for l in range(NL):
                nc.sync.dma_start(ebias_sb[:, l * RLOC:(l + 1) * RLOC], ebias[l])

            # collective bounce buffers (DRAM pool so Tile tracks deps)
            ar_in = [dramp.tile([B, S, FE], dt.float32, name=f"ar_in{l}")
                     for l in range(NL)]
            ar_out = dramp.tile([B, S, FE], dt.float32, name="ar_out",
                                addr_space="Shared")
            rs_out = dramp.tile([S, FE], dt.float32, name="rs_out")

            # layer-1 lhsT: x^T per batch, [128(i within chunk), 6*128(s)]
            hT = []
            for bb in range(B):
                t = hTp.tile([128, IC * S], dt.bfloat16, name=f"hT{bb}", tag="hT")
                nc.sync.dma_start(
                    t[:].rearrange("p (c s) -> p c s", c=IC),
                    xt[bb].rearrange("(c p) s -> p c s", p=128),
                )
                hT.append(t)

            payload = [
                [paylp.tile([S, FE], dt.float32, name=f"pay{l}_{bb}", tag=f"pay{l}")
                 for bb in range(B)]
                for l in range(NL)
            ]

            def rgcn_layer(l, hT_tiles):
                denacc = tailp.tile([S, B], dt.float32, name=f"denacc{l}",
                                    tag="denacc")
                for r in range(RLOC):
                    wt = []
                    for ic in range(IC):
                        t = wpool.tile([128, FE], dt.bfloat16,
                                       name=f"w{l}_{r}_{ic}", tag="wt")
                        nc.sync.dma_start(t[:], w[l, r, ic * 128:(ic + 1) * 128, :])
                        wt.append(t)
                    ld_ps = ps_ld.tile([128, B, 4], dt.float32,
                                       name=f"ld{l}_{r}", tag="ld")
                    adjT_l, hid_l = [], []
                    for bb in range(B):
                        adjT = adjp.tile([S, S], dt.bfloat16,
                                         name=f"adjT{l}_{r}_{bb}", tag="adjT")
                        nc.sync.dma_start(adjT[:], adjt[r, bb])
                        hid_ps = ps_hid.tile([S, FE], dt.float32,
                                             name=f"hps{l}_{r}_{bb}", tag="hid")
                        for ic in range(IC):
                            lhsT = hT_tiles[bb][:, ic * S:(ic + 1) * S]
                            # psum-bank-sized output regions
                            nc.tensor.matmul(
                                hid_ps[:, 0:512], lhsT=lhsT,
                                rhs=wt[ic][:, 0:512],
                                start=(ic == 0), stop=(ic == IC - 1))
                            nc.tensor.matmul(
                                hid_ps[:, 512:FE], lhsT=lhsT,
                                rhs=wt[ic][:, 512:FE],
                                start=(ic == 0), stop=(ic == IC - 1))
                        hid = hidp.tile([S, FE + 1], dt.bfloat16,
                                        name=f"hid{l}_{r}_{bb}", tag="hid")
                        nc.scalar.copy(hid[:, :FE], hid_ps[:])
                        nc.gpsimd.memset(hid[:, FE:FE + 1], 1.0)
                        # [logun | denom] column pair for batch bb
                        nc.tensor.matmul(
                            ld_ps[:, bb, 0:2],
                            lhsT=adjT[:],
                            rhs=hid[:, F:FE + 1],
                            start=True, stop=True,
                        )
                        adjT_l.append(adjT)
                        hid_l.append(hid)
                    # ---- tail for slot r (batched over b) ----
                    dsafe = tailp.tile([S, B], dt.float32, name=f"ds{l}{r}", tag="ds")
                    nc.vector.tensor_scalar_max(dsafe[:], ld_ps[:, :, 1], 1e-30)
                    rec = tailp.tile([S, B], dt.float32, name=f"rc{l}{r}", tag="rc")
                    nc.vector.reciprocal(rec[:], dsafe[:])
                    tmul = tailp.tile([S, B], dt.float32, name=f"tm{l}{r}", tag="tm")
                    nc.vector.tensor_mul(tmul[:], ld_ps[:, :, 0], rec[:])
                    ee = tailp.tile([S, B], dt.float32, name=f"ee{l}{r}", tag="ee")
                    nc.scalar.activation(ee[:], tmul[:], AF.Exp,
                                         bias=ebias_sb[:, l * RLOC + r:
                                                       l * RLOC + r + 1])
                    scr = tailp.tile([S, B], dt.float32, name=f"sc{l}{r}", tag="sc")
                    nc.vector.tensor_mul(scr[:], ee[:], rec[:])
                    if r == 0:
                        nc.vector.tensor_copy(denacc[:], ee[:])
                    else:
                        nc.vector.tensor_add(denacc[:], denacc[:], ee[:])
                    # ---- aggregation + weighted accumulation ----
                    for bb in range(B):
                        for half in range(2):
                            c0 = half * 384
                            intm = ps_intm.tile([S, 384], dt.float32,
                                                name=f"in{l}{r}{bb}{half}",
                                                tag="intm")
                            nc.tensor.matmul(
                                intm[:],
                                lhsT=adjT_l[bb][:],
                                rhs=hid_l[bb][:, c0:c0 + 384],
                                start=True, stop=True,
                            )
                            dst = payload[l][bb][:, c0:c0 + 384]
                            if r == 0:
                                nc.vector.tensor_scalar(
                                    dst, intm[:], scr[:, bb:bb + 1], None,
                                    OP.mult)
                            else:
                                nc.vector.scalar_tensor_tensor(
                                    dst, intm[:], scr[:, bb:bb + 1], dst,
                                    OP.mult, OP.add)
                for bb in range(B):
                    nc.vector.tensor_copy(payload[l][bb][:, F:FE],
                                          denacc[:, bb:bb + 1])

            # =================== layer 1 ===================
            rgcn_layer(0, hT)
            for bb in range(B):
                nc.sync.dma_start(ar_in[0][bb], payload[0][bb][:])
            nc.gpsimd.collective_compute(
                "AllReduce", OP.add, replica_groups=groups,
                ins=[ar_in[0].opt()], outs=[ar_out.opt()],
            )
            # h2 = relu(num/den); build h2^T as layer-2 lhsT
            h2T = []
            if True:
                for bb in range(B):
                    raw = miscp.tile([S, FE], dt.float32, name=f"raw{bb}",
                                     tag="raw")
                    nc.sync.dma_start(raw[:], ar_out[bb])
                    rd = miscp.tile([S, 1], dt.float32, name=f"rd{bb}", tag="rd")
                    nc.vector.reciprocal(rd[:], raw[:, F:FE])
                    h2 = miscp.tile([S, F], dt.bfloat16, name=f"h2_{bb}",
                                    tag="h2")
                    nc.scalar.activation(h2[:], raw[:, :F], AF.Relu, scale=rd[:])
                    t = hTp.tile([128, IC * S], dt.bfloat16, name=f"h2T{bb}",
                                 tag="hT")
                    for ic in range(IC):
                        tp = ps_ld.tile([128, 128], dt.bfloat16,
                                        name=f"tp{bb}_{ic}", tag="ld")
                        nc.tensor.transpose(tp[:], h2[:, ic * 128:(ic + 1) * 128],
                                            ident_b[:])
                        nc.scalar.copy(t[:, ic * S:(ic + 1) * S], tp[:])
                    h2T.append(t)

                # =================== layer 2 ===================
                rgcn_layer(1, h2T)
                for bb in range(B):
                    nc.sync.dma_start(ar_in[1][bb], payload[1][bb][:])
                nc.gpsimd.collective_compute(
                    "ReduceScatter", OP.add, replica_groups=groups,
                    ins=[ar_in[1].opt()], outs=[rs_out.opt()],
                )

                # =================== attention (batch = core id) ==========
                raw = miscp.tile([S, FE], dt.float32, name="rawf", tag="raw")
                nc.sync.dma_start(raw[:], rs_out[:])
                rd = miscp.tile([S, 1], dt.float32, name="rdf", tag="rd")
                nc.vector.reciprocal(rd[:], raw[:, F:FE])
                hf = miscp.tile([S, F], dt.bfloat16, name="hf", tag="h2")
                nc.scalar.activation(hf[:], raw[:, :F], AF.Relu, scale=rd[:])
                hfT = hTp.tile([128, IC * S], dt.bfloat16, name="hfT", tag="hT")
                for ic in range(IC):
                    tp = ps_ld.tile([128, 128], dt.bfloat16, name=f"ftp{ic}",
                                    tag="ld")
                    nc.tensor.transpose(tp[:], hf[:, ic * 128:(ic + 1) * 128],
                                        ident_b[:])
                    nc.scalar.copy(hfT[:, ic * S:(ic + 1) * S], tp[:])

                # kx = hf @ wk + bk   [S, 768]
                kx_ps = ps_hid.tile([S, F], dt.float32, name="kx_ps", tag="hid")
                bk_sb = constp.tile([1, F], dt.bfloat16, name="bk_sb")
                nc.sync.dma_start(bk_sb[:], bk_d[:])
                for ic in range(IC):
                    wkt = wpool.tile([128, F], dt.bfloat16, name=f"wk{ic}",
                                     tag="wt")
                    nc.sync.dma_start(wkt[:], wk_d[ic * 128:(ic + 1) * 128, :])
                    lhsT = hfT[:, ic * S:(ic + 1) * S]
                    nc.tensor.matmul(kx_ps[:, 0:512], lhsT=lhsT,
                                     rhs=wkt[:, 0:512],
                                     start=(ic == 0), stop=False)
                    nc.tensor.matmul(kx_ps[:, 512:F], lhsT=lhsT,
                                     rhs=wkt[:, 512:F],
                                     start=(ic == 0), stop=False)
                nc.tensor.matmul(kx_ps[:, 0:512], lhsT=ones_row[:],
                                 rhs=bk_sb[:, 0:512], start=False, stop=True)
                nc.tensor.matmul(kx_ps[:, 512:F], lhsT=ones_row[:],
                                 rhs=bk_sb[:, 512:F], start=False, stop=True)
                kx = miscp.tile([S, F], dt.bfloat16, name="kx", tag="h2")
                nc.scalar.copy(kx[:], kx_ps[:])
                # kxT per head: [96, 8*128]
                kxT = miscp.tile([HD, NH * S], dt.bfloat16, name="kxT",
                                 tag="kxT")
                for hh in range(NH):
                    tp = ps_ld.tile([128, 128], dt.bfloat16, name=f"ktp{hh}",
                                    tag="ld")
                    nc.tensor.transpose(tp[:HD, :], kx[:, hh * HD:(hh + 1) * HD],
                                        ident_b[:])
                    nc.scalar.copy(kxT[:, hh * S:(hh + 1) * S], tp[:HD, :])

                # qx = q @ wq + bq    [1, 768]
                qx_ps = ps_intm.tile([1, 512], dt.float32, name="qx_ps",
                                     tag="intm")
                qx_ps2 = ps_intm.tile([1, 256], dt.float32, name="qx_ps2",
                                      tag="intm")
                qc = []
                for ic in range(IC):
                    t = constp.tile([S, 1], dt.bfloat16, name=f"qc{ic}")
                    nc.sync.dma_start(t[:], qcol_d[ic])
                    qc.append(t)
                bq_sb = constp.tile([1, F], dt.bfloat16, name="bq_sb")
                nc.sync.dma_start(bq_sb[:], bq_d[:])
                for ic in range(IC):
                    wqt = wpool.tile([128, F], dt.bfloat16, name=f"wq{ic}",
                                     tag="wt")
                    nc.sync.dma_start(wqt[:], wq_d[ic * 128:(ic + 1) * 128, :])
                    nc.tensor.matmul(qx_ps[:], lhsT=qc[ic][:],
                                     rhs=wqt[:, 0:512],
                                     start=(ic == 0), stop=False)
                    nc.tensor.matmul(qx_ps2[:], lhsT=qc[ic][:],
                                     rhs=wqt[:, 512:F],
                                     start=(ic == 0), stop=False)
                nc.tensor.matmul(qx_ps[:], lhsT=one_sb[:], rhs=bq_sb[:, 0:512],
                                 start=False, stop=True)
                nc.tensor.matmul(qx_ps2[:], lhsT=one_sb[:], rhs=bq_sb[:, 512:F],
                                 start=False, stop=True)
                qx = miscp.tile([1, F], dt.bfloat16, name="qx", tag="qx")
                nc.scalar.copy(qx[:, 0:512], qx_ps[:])
                nc.scalar.copy(qx[:, 512:F], qx_ps2[:])
                # qxT per head as columns [96, 8]
                qxT_ps = ps_ld.tile([HD, NH, 4], dt.bfloat16, name="qxT_ps",
                                    tag="ld")
                for hh in range(NH):
                    nc.tensor.transpose(qxT_ps[:, hh, 0:1],
                                        qx[:, hh * HD:(hh + 1) * HD],
                                        ident_b[:1, :1])
                qxT = miscp.tile([HD, NH], dt.bfloat16, name="qxT", tag="qx")
                nc.scalar.copy(qxT[:], qxT_ps[:, :, 0])
                # qwT[h] = wbil^T @ qxT[h]  -> [96, 8]
                wbil_sb = constp.tile([HD, HD], dt.bfloat16, name="wbil_sb")
                nc.sync.dma_start(wbil_sb[:], wbil_d[:])
                qw_ps = ps_intm.tile([HD, NH, 4], dt.float32, name="qw_ps",
                                     tag="intm")
                for hh in range(NH):
                    nc.tensor.matmul(qw_ps[:, hh, 0:1], lhsT=wbil_sb[:],
                                     rhs=qxT[:, hh:hh + 1], start=True,
                                     stop=True)
                qwT = miscp.tile([HD, NH], dt.bfloat16, name="qwT", tag="qx")
                nc.scalar.copy(qwT[:], qw_ps[:, :, 0])
                # scoreT[:,h] = kx_h @ qwT_h    [128, 8]
                sc_ps = ps_intm.tile([S, NH, 4], dt.float32, name="sc_ps",
                                     tag="intm")
                for hh in range(NH):
                    nc.tensor.matmul(sc_ps[:, hh, 0:1],
                                     lhsT=kxT[:, hh * S:(hh + 1) * S],
                                     rhs=qwT[:, hh:hh + 1], start=True,
                                     stop=True)
                sc_sb = miscp.tile([S, NH], dt.float32, name="sc_sb", tag="scb")
                nc.scalar.copy(sc_sb[:], sc_ps[:, :, 0])
                # score rows [8, 128]
                srow_ps = ps_ld.tile([NH, S], dt.float32, name="srow", tag="ld")
                nc.tensor.transpose(srow_ps[:], sc_sb[:], ident_f[:])
                negmax = miscp.tile([NH, 1], dt.float32, name="negmax", tag="sm")
                nc.vector.tensor_reduce(negmax[:], srow_ps[:],
                                        mybir.AxisListType.X, OP.max,
                                        negate=True)
                esc = miscp.tile([NH, S], dt.float32, name="esc", tag="esc")
                sumexp = miscp.tile([NH, 1], dt.float32, name="sumexp", tag="sm")
                nc.scalar.activation(esc[:], srow_ps[:], AF.Exp, bias=negmax[:],
                                     accum_out=sumexp[:])
                rsm = miscp.tile([NH, 1], dt.float32, name="rsm", tag="sm")
                nc.vector.reciprocal(rsm[:], sumexp[:])
                attn = miscp.tile([NH, S], dt.bfloat16, name="attn", tag="esc")
                nc.vector.tensor_scalar_mul(attn[:], esc[:], rsm[:])
                # attnT [128, 8]
                at_ps = ps_ld.tile([S, NH], dt.bfloat16, name="at_ps", tag="ld")
                nc.tensor.transpose(at_ps[:], attn[:], ident_b[:NH, :NH])
                attnT = miscp.tile([S, NH], dt.bfloat16, name="attnT", tag="scb")
                nc.scalar.copy(attnT[:], at_ps[:])
                # o[0, h*96:(h+1)*96] = attn_h @ kx_h  (bank-safe [1,8,128] tile)
                o_psA = ps_intm.tile([1, 4, 128], dt.float32, name="o_psA",
                                     tag="intm")
                o_psB = ps_intm.tile([1, 4, 128], dt.float32, name="o_psB",
                                     tag="intm")
                for hh in range(NH):
                    tgt = o_psA if hh < 4 else o_psB
                    nc.tensor.matmul(tgt[:, hh % 4, :HD],
                                     lhsT=attnT[:, hh:hh + 1],
                                     rhs=kx[:, hh * HD:(hh + 1) * HD],
                                     start=True, stop=True)
                o_sb = miscp.tile([1, F], dt.bfloat16, name="o_sb", tag="qx")
                nc.scalar.copy(o_sb[:, 0:384], o_psA[:, :, :HD])
                nc.scalar.copy(o_sb[:, 384:F], o_psB[:, :, :HD])
                # oT [128, 6]
                oT_ps = ps_ld.tile([S, IC, 4], dt.bfloat16, name="oT_ps",
                                    tag="ld")
                for ic in range(IC):
                    nc.tensor.transpose(oT_ps[:, ic, 0:1],
                                        o_sb[:, ic * 128:(ic + 1) * 128],
                                        ident_b[:1, :1])
                oT = miscp.tile([S, IC], dt.bfloat16, name="oT", tag="scb")
                nc.scalar.copy(oT[:], oT_ps[:, :, 0])
                # res = o @ wproj + bproj
                res_ps = ps_hid.tile([1, 512], dt.float32, name="res_ps",
                                     tag="hid")
                res_ps2 = ps_hid.tile([1, 256], dt.float32, name="res_ps2",
                                      tag="hid")
                bp_sb = constp.tile([1, F], dt.bfloat16, name="bp_sb")
                nc.sync.dma_start(bp_sb[:], bproj_d[:])
                for ic in range(IC):
                    wpt = wpool.tile([128, F], dt.bfloat16, name=f"wp{ic}",
                                     tag="wt")
                    nc.sync.dma_start(wpt[:], wproj_d[ic * 128:(ic + 1) * 128, :])
                    nc.tensor.matmul(res_ps[:], lhsT=oT[:, ic:ic + 1],
                                     rhs=wpt[:, 0:512],
                                     start=(ic == 0), stop=False)
                    nc.tensor.matmul(res_ps2[:], lhsT=oT[:, ic:ic + 1],
                                     rhs=wpt[:, 512:F],
                                     start=(ic == 0), stop=False)
                nc.tensor.matmul(res_ps[:], lhsT=one_sb[:], rhs=bp_sb[:, 0:512],
                                 start=False, stop=True)
                nc.tensor.matmul(res_ps2[:], lhsT=one_sb[:], rhs=bp_sb[:, 512:F],
                                 start=False, stop=True)
                res_sb = miscp.tile([1, F], dt.float32, name="res_sb", tag="res")
                nc.scalar.copy(res_sb[:, 0:512], res_ps[:])
                nc.scalar.copy(res_sb[:, 512:F], res_ps2[:])
                nc.sync.dma_start(out_d[:], res_sb[:])

    nc.compile()
    _CACHE["nc"] = nc
    return nc


def _prep_inputs(x, adj, q, w_rgcn, score_w, score_b, wk, bk, wq, bq, wbil,
                 wproj, bproj):
    f32 = np.float32
    x = np.asarray(x, f32)
    adj = np.asarray(adj, f32)
    q = np.asarray(q, f32)
    w_rgcn = np.asarray(w_rgcn, f32)
    score_w = np.asarray(score_w, f32)
    score_b = np.asarray(score_b, f32)

    u = np.einsum("lrio,lo->lri", w_rgcn, score_w).astype(f32)
    w_ext = np.concatenate([w_rgcn, u[..., None]], axis=-1)  # [2,41,768,769]

    xt_all = np.ascontiguousarray(x.transpose(0, 2, 1)).astype(bf16)  # [B,F,S]

    shared = {
        "xt": xt_all,
        "wk": np.asarray(wk, f32).astype(bf16),
        "wq": np.asarray(wq, f32).astype(bf16),
        "wbil": np.asarray(wbil, f32).astype(bf16),
        "wproj": np.asarray(wproj, f32).astype(bf16),
        "bk": np.asarray(bk, f32).reshape(1, F).astype(bf16),
        "bq": np.asarray(bq, f32).reshape(1, F).astype(bf16),
        "bproj": np.asarray(bproj, f32).reshape(1, F).astype(bf16),
    }

    in_maps = []
    for c in range(NCORES):
        nreal = max(0, min(R - c * RLOC, RLOC))
        w_c = np.zeros((NL, RLOC, F, FE), f32)
        adjt_c = np.zeros((RLOC, B, S, S), f32)
        eb_c = np.full((NL, S, RLOC), EBIAS_MASK, f32)
        if nreal > 0:
            sl = slice(c * RLOC, c * RLOC + nreal)
            w_c[:, :nreal] = w_ext[:, sl]
            # adjt[r, b] = adj[b, g].T
            adjt_c[:nreal] = adj[:, sl].transpose(1, 0, 3, 2)
            for l in range(NL):
                eb_c[l, :, :nreal] = score_b[l]
        m = dict(shared)
        m["w"] = w_c.astype(bf16)
        m["adjt"] = np.ascontiguousarray(adjt_c).astype(bf16)
        m["ebias"] = eb_c
        m["qcol"] = q[c].reshape(IC, S, 1).astype(bf16)
        in_maps.append(m)
    return in_maps


def kernel(**inputs) -> np.ndarray:
    from concourse.bass_utils import run_bass_kernel_spmd

    nc = _build_graph()
    in_maps = _prep_inputs(**inputs)
    res = run_bass_kernel_spmd(nc, in_maps, core_ids=list(range(NCORES)))
    outs = [np.asarray(res.results[c]["out"], np.float32) for c in range(NCORES)]
    return np.stack(outs)  # [8, 1, 768]
